# revision 21
# baseline (speedup 1.0000x reference)
# SAGAN self-attention (B=4, H=W=64, C=64, D=8) on 8 TRN2 NeuronCores.
#
# Sharding: core i = (batch b=i//2, half h=i%2). Each core computes rows
# [h*2048, (h+1)*2048) of the 4096x4096 attention for its batch, fully fused
# in SBUF (no NxN matrix ever touches HBM, no collectives).
#
# Scores are computed TRANSPOSED, sT[m, n] (keys m on partitions, queries n
# on free axis) directly from x: s = x @ (Wf Wg^T) @ x^T, so the contraction
# is K=64 (channels) rather than K=8 (qk dim). The contraction is then
# zero-padded to K=128: the PE clock-gate (HAM) only counts the PE as
# "busy" when all 128 rows are active, so K<128 matmuls run at the cold
# 1.2 GHz clock forever (measured: 629 ns vs 379 ns per 512-col matmul).
# Pad row 64 carries the bias term bg.(f_m + bf) (with a ones-row in the
# moving operand), which softmax needs; the row-constant term g_n.bf is
# softmax-invariant and dropped.
#
# exp(sT) feeds the PV matmul directly as the moving operand. The PV
# stationary is [hv(8) | 0.. | 1@32], so one accumulating K=128 matmul
# produces v_unnorm^T [8, n] AND the softmax denominator on PSUM partition
# 32 (a legal 32-aligned read base). The epilogue packs v_un^T (rows 0-7)
# and the denominator (row 32) into one zero-padded K=128 stationary VD2:
# rhs=WV2 (Wv rows 0-7) gives o_un, rhs=E8 (one-hot row 32) transposes the
# denominator to a [128,1] column for the reciprocal. o = o_un/denom fused
# with the residual add as one scalar_tensor_tensor per [128, 64] tile.
#
# Host precomputes the tiny projections (0.6% of FLOPs) and folds:
# P = Wf@Wg^T into the score form, gamma into Wv, gamma*(bh@Wv+bv) into the
# residual. Attention matmuls use bf16 operands (fp32/f32r moving operands
# stream at 2-3 cycles/column vs 1 for bf16); PSUM accumulation is fp32.

import numpy as np
import ml_dtypes

import concourse.bacc as bacc
import concourse.tile as tile
import concourse.mybir as mybir
from concourse.alu_op_type import AluOpType
from concourse.bass_utils import run_bass_kernel_spmd

F32 = mybir.dt.float32
BF16 = mybir.dt.bfloat16
AFT = mybir.ActivationFunctionType

B, HH, WW, C = 4, 64, 64, 64
N = HH * WW          # 4096 sequence positions per batch
D = 8                # qkv channel dim
RPC = N // 2         # rows per core (2048)
NCORES = 8
HW9 = 33             # PV stationary width: [hv(8) | zeros(24) | ones(1)]
MC = N // 128        # 32 key chunks of 128


def _build():
    nc = bacc.Bacc("TRN2", target_bir_lowering=False, debug=False,
                   num_devices=NCORES)

    xt2 = nc.dram_tensor("xt2", [128, N], BF16, kind="ExternalInput").ap()
    gp2 = nc.dram_tensor("gp2", [128, RPC], BF16, kind="ExternalInput").ap()
    hvo = nc.dram_tensor("hvo", [128, MC * HW9], BF16,
                         kind="ExternalInput").ap()
    xrp = nc.dram_tensor("xrp", [128, RPC // 128 * C], F32,
                         kind="ExternalInput").ap()
    wv2 = nc.dram_tensor("wv2", [128, C], BF16, kind="ExternalInput").ap()
    out = nc.dram_tensor("out", [RPC, C], F32, kind="ExternalOutput").ap()

    NT = 4            # n-tiles of 512 query rows each
    TN = 512          # queries per n-tile
    SW = [3] * 10 + [2]   # swath sizes (in key chunks) per n-tile

    with tile.TileContext(nc) as tc:
        with tc.tile_pool(name="const", bufs=1) as const:
            XT2 = const.tile([128, N], BF16)
            GP2 = const.tile([128, RPC], BF16)
            HVO = const.tile([128, MC * HW9], BF16)
            XRP = const.tile([128, RPC // 128 * C], F32)
            WV2 = const.tile([128, C], BF16)
            E8 = const.tile([128, 1], BF16)
            PRE = const.tile([1, 1], F32)
            WUP = const.tile([128, 256], BF16)

            # interleave input DMAs in first-use order: swath s of n-tile 0
            # needs XT2 chunks, GP2 cols 0:512, and HVO chunks as m advances
            nc.sync.dma_start(GP2[:, 0:512], gp2[:, 0:512])
            w8 = MC * HW9 // 8
            for j in range(8):
                nc.sync.dma_start(XT2[:, j * 512:(j + 1) * 512],
                                  xt2[:, j * 512:(j + 1) * 512])
                nc.sync.dma_start(HVO[:, j * w8:(j + 1) * w8],
                                  hvo[:, j * w8:(j + 1) * w8])
            for j in range(1, 4):
                nc.sync.dma_start(GP2[:, j * 512:(j + 1) * 512],
                                  gp2[:, j * 512:(j + 1) * 512])
            nc.sync.dma_start(WV2[:], wv2[:])
            nc.sync.dma_start(XRP[:], xrp[:])
            nc.vector.memset(E8[:], 0.0)
            nc.vector.memset(E8[32:33, :], 1.0)
            # dummy exp: hoists the one-time ACT table load (~1.3us) into the
            # initial DMA wait instead of the first swath's critical path
            nc.scalar.activation(PRE[:], E8[0:1, :], AFT.Exp)
            # PE warm-up: the HAM clock gate needs ~3.4us of full-K activity
            # before it lifts the 1.2->2.4 GHz throttle; burn the initial DMA
            # wait on dummy K=128 matmuls so real swaths start warm
            nc.vector.memset(WUP[:], 0.0)

            with tc.tile_pool(name="ps_s", bufs=2, space="PSUM") as ps_s, \
                 tc.tile_pool(name="ps_v", bufs=1, space="PSUM") as ps_vp, \
                 tc.tile_pool(name="ps_e", bufs=1, space="PSUM") as ps_ep, \
                 tc.tile_pool(name="expp", bufs=4) as expp, \
                 tc.tile_pool(name="vd2p", bufs=2) as vd2p, \
                 tc.tile_pool(name="scolp", bufs=2) as scolp, \
                 tc.tile_pool(name="osbp", bufs=4) as osbp:
                for wi in range(16):
                    wps = ps_ep.tile([128, 260], F32, tag="pse")
                    nc.tensor.matmul(wps[:, 0:256], lhsT=WUP[:, 0:128],
                                     rhs=WUP[:], start=True, stop=True)
                for nt in range(NT):
                    n0 = nt * TN
                    psv = ps_vp.tile([33, TN], F32)
                    vd2 = vd2p.tile([128, TN], BF16)
                    nc.vector.memset(vd2[:], 0.0)
                    m = 0
                    for sw in SW:
                        ps = ps_s.tile([128, 1536], F32)
                        ex = expp.tile([128, 1536], BF16)
                        w = sw * 512
                        for k in range(sw):
                            nc.tensor.matmul(
                                ps[:, k * 512:(k + 1) * 512],
                                lhsT=XT2[:, (m + k) * 128:(m + k + 1) * 128],
                                rhs=GP2[:, n0:n0 + TN],
                                start=True, stop=True)
                        nc.scalar.activation(ex[:, :w], ps[:, :w], AFT.Exp)
                        for k in range(sw):
                            nc.tensor.matmul(
                                psv[:],
                                lhsT=HVO[:, (m + k) * HW9:(m + k + 1) * HW9],
                                rhs=ex[:, k * 512:(k + 1) * 512],
                                start=(m + k == 0), stop=(m + k == MC - 1),
                                skip_group_check=True)
                        m += sw
                    nc.vector.tensor_copy(vd2[0:D, :], psv[0:D, :])
                    nc.vector.tensor_copy(vd2[32:33, :], psv[32:33, :])
                    pse = ps_ep.tile([128, 260], F32, tag="pse")
                    scol = scolp.tile([128, 4], F32)
                    for nb in range(4):
                        nc.tensor.matmul(pse[:, 256 + nb:257 + nb],
                                         lhsT=vd2[:, nb * 128:(nb + 1) * 128],
                                         rhs=E8[:], start=True, stop=True)
                    for nb in range(4):
                        nc.tensor.matmul(pse[:, nb * 64:(nb + 1) * 64],
                                         lhsT=vd2[:, nb * 128:(nb + 1) * 128],
                                         rhs=WV2[:], start=True, stop=True)
                    nc.vector.reciprocal(scol[:], pse[:, 256:260])
                    for nb in range(4):
                        osb = osbp.tile([128, C], F32)
                        t = nt * 4 + nb
                        nc.vector.scalar_tensor_tensor(
                            osb[:], pse[:, nb * 64:(nb + 1) * 64],
                            scol[:, nb:nb + 1],
                            XRP[:, t * C:(t + 1) * C],
                            op0=AluOpType.mult, op1=AluOpType.add)
                        nc.sync.dma_start(out[t * 128:(t + 1) * 128, :], osb[:])

    nc.compile()
    return nc


_CACHE = {}


def _get_compiled():
    if "nc" not in _CACHE:
        _CACHE["nc"] = _build()
    return _CACHE["nc"]


def _make_in_maps(x, Wf, bf, Wg, bg, Wh, bh, Wv, bv, gamma):
    x = np.asarray(x, np.float32)
    Wf = np.asarray(Wf, np.float32)
    Wg = np.asarray(Wg, np.float32)
    Wh = np.asarray(Wh, np.float32)
    Wv = np.asarray(Wv, np.float32)
    bf = np.asarray(bf, np.float32)
    bg = np.asarray(bg, np.float32)
    bh = np.asarray(bh, np.float32)
    bv = np.asarray(bv, np.float32)
    g0 = float(np.asarray(gamma, np.float32).reshape(-1)[0])

    xf = x.reshape(B, N, C)
    P = Wf @ Wg.T                            # [C, C] score kernel
    wfbg = Wf @ bg                           # [C] column-bias direction
    bgbf = float(bg @ bf)
    res_bias = g0 * (bh @ Wv + bv)           # [C] folded into residual
    wv2 = np.zeros((128, C), np.float32)
    wv2[0:D] = g0 * Wv
    wv2 = wv2.astype(ml_dtypes.bfloat16)

    in_maps = []
    for i in range(NCORES):
        b, h = divmod(i, 2)
        r0 = h * RPC
        xt2 = np.zeros((128, N), np.float32)
        xt2[0:C] = xf[b].T
        xt2[C] = xf[b] @ wfbg + bgbf         # d_m: per-key score bias
        gp2 = np.zeros((128, RPC), np.float32)
        gp2[0:C] = P @ xf[b, r0:r0 + RPC].T
        gp2[C] = 1.0
        hv = xf[b] @ Wh                      # [N, D] (bh folds into res_bias)
        ho = np.zeros((MC, 128, HW9), np.float32)
        ho[:, :, 0:D] = hv.reshape(MC, 128, D)
        ho[:, :, 32] = 1.0
        ho = np.ascontiguousarray(ho.transpose(1, 0, 2).reshape(128, -1))
        xr = xf[b, r0:r0 + RPC] + res_bias   # [RPC, C]
        xrp = np.ascontiguousarray(
            xr.reshape(RPC // 128, 128, C).transpose(1, 0, 2).reshape(128, -1))
        in_maps.append({"xt2": xt2.astype(ml_dtypes.bfloat16),
                        "gp2": gp2.astype(ml_dtypes.bfloat16),
                        "hvo": ho.astype(ml_dtypes.bfloat16),
                        "xrp": xrp, "wv2": wv2})
    return in_maps


def _assemble(results):
    outf = np.empty((B, N, C), np.float32)
    for i in range(NCORES):
        b, h = divmod(i, 2)
        outf[b, h * RPC:(h + 1) * RPC] = results[i]["out"]
    return outf.reshape(B, HH, WW, C)


def run(inputs, **spmd_kwargs):
    """Returns (output, BassKernelResults)."""
    nc = _get_compiled()
    in_maps = _make_in_maps(**inputs)
    res = run_bass_kernel_spmd(nc, in_maps, core_ids=list(range(NCORES)),
                               **spmd_kwargs)
    return _assemble(res.results), res


def kernel(**inputs):
    out, _ = run(inputs)
    return out


# revision 22
# speedup vs baseline: 1.0238x; 1.0238x over previous
# SAGAN self-attention (B=4, H=W=64, C=64, D=8) on 8 TRN2 NeuronCores.
#
# Sharding: core i = (batch b=i//2, half h=i%2). Each core computes rows
# [h*2048, (h+1)*2048) of the 4096x4096 attention for its batch, fully fused
# in SBUF (no NxN matrix ever touches HBM, no collectives).
#
# Scores are computed TRANSPOSED, sT[m, n] (keys m on partitions, queries n
# on free axis) directly from x: s = x @ (Wf Wg^T) @ x^T, so the contraction
# is K=64 (channels) rather than K=8 (qk dim). The contraction is then
# zero-padded to K=128: the PE clock-gate (HAM) only counts the PE as
# "busy" when all 128 rows are active, so K<128 matmuls run at the cold
# 1.2 GHz clock forever (measured: 629 ns vs 379 ns per 512-col matmul).
# Pad row 64 carries the bias term bg.(f_m + bf) (with a ones-row in the
# moving operand), which softmax needs; the row-constant term g_n.bf is
# softmax-invariant and dropped.
#
# exp(sT) feeds the PV matmul directly as the moving operand. The PV
# stationary is [hv(8) | 0.. | 1@32], so one accumulating K=128 matmul
# produces v_unnorm^T [8, n] AND the softmax denominator on PSUM partition
# 32 (a legal 32-aligned read base). The epilogue packs v_un^T (rows 0-7)
# and the denominator (row 32) into one zero-padded K=128 stationary VD2:
# rhs=WV2 (Wv rows 0-7) gives o_un, rhs=E8 (one-hot row 32) transposes the
# denominator to a [128,1] column for the reciprocal. o = o_un/denom fused
# with the residual add as one scalar_tensor_tensor per [128, 64] tile.
#
# Host precomputes the tiny projections (0.6% of FLOPs) and folds:
# P = Wf@Wg^T into the score form, gamma into Wv, gamma*(bh@Wv+bv) into the
# residual. Attention matmuls use bf16 operands (fp32/f32r moving operands
# stream at 2-3 cycles/column vs 1 for bf16); PSUM accumulation is fp32.

import numpy as np
import ml_dtypes

import concourse.bacc as bacc
import concourse.tile as tile
import concourse.mybir as mybir
from concourse.alu_op_type import AluOpType
from concourse.bass_utils import run_bass_kernel_spmd

F32 = mybir.dt.float32
BF16 = mybir.dt.bfloat16
AFT = mybir.ActivationFunctionType

B, HH, WW, C = 4, 64, 64, 64
N = HH * WW          # 4096 sequence positions per batch
D = 8                # qkv channel dim
RPC = N // 2         # rows per core (2048)
NCORES = 8
HW9 = 33             # PV stationary width: [hv(8) | zeros(24) | ones(1)]
MC = N // 128        # 32 key chunks of 128


def _build():
    nc = bacc.Bacc("TRN2", target_bir_lowering=False, debug=False,
                   num_devices=NCORES)

    xt2 = nc.dram_tensor("xt2", [128, N], BF16, kind="ExternalInput").ap()
    gp2 = nc.dram_tensor("gp2", [128, RPC], BF16, kind="ExternalInput").ap()
    hvo = nc.dram_tensor("hvo", [128, MC * HW9], BF16,
                         kind="ExternalInput").ap()
    xrp = nc.dram_tensor("xrp", [128, RPC // 128 * C], F32,
                         kind="ExternalInput").ap()
    wv2 = nc.dram_tensor("wv2", [128, C], BF16, kind="ExternalInput").ap()
    out = nc.dram_tensor("out", [RPC, C], F32, kind="ExternalOutput").ap()

    NT = 4            # n-tiles of 512 query rows each
    TN = 512          # queries per n-tile
    SW = [3] * 10 + [2]   # swath sizes (in key chunks) per n-tile

    with tile.TileContext(nc) as tc:
        with tc.tile_pool(name="const", bufs=1) as const:
            XT2 = const.tile([128, N], BF16)
            GP2 = const.tile([128, RPC], BF16)
            HVO = const.tile([128, MC * HW9], BF16)
            XRP = const.tile([128, RPC // 128 * C], F32)
            WV2 = const.tile([128, C], BF16)
            E8 = const.tile([128, 1], BF16)
            PRE = const.tile([1, 1], F32)
            WUP = const.tile([128, 256], BF16)

            # interleave input DMAs in first-use order: swath s of n-tile 0
            # needs XT2 chunks, GP2 cols 0:512, and HVO chunks as m advances
            nc.sync.dma_start(GP2[:, 0:512], gp2[:, 0:512])
            w8 = MC * HW9 // 8
            for j in range(8):
                nc.sync.dma_start(XT2[:, j * 512:(j + 1) * 512],
                                  xt2[:, j * 512:(j + 1) * 512])
                nc.sync.dma_start(HVO[:, j * w8:(j + 1) * w8],
                                  hvo[:, j * w8:(j + 1) * w8])
            for j in range(1, 4):
                nc.sync.dma_start(GP2[:, j * 512:(j + 1) * 512],
                                  gp2[:, j * 512:(j + 1) * 512])
            nc.sync.dma_start(WV2[:], wv2[:])
            nc.sync.dma_start(XRP[:], xrp[:])
            nc.vector.memset(WUP[:], 0.0)
            nc.vector.memset(E8[:], 0.0)
            nc.vector.memset(E8[32:33, :], 1.0)
            # dummy exp: hoists the one-time ACT table load (~1.3us) into the
            # initial DMA wait instead of the first swath's critical path
            nc.scalar.activation(PRE[:], E8[0:1, :], AFT.Exp)

            with tc.tile_pool(name="ps_s", bufs=2, space="PSUM") as ps_s, \
                 tc.tile_pool(name="ps_v", bufs=1, space="PSUM") as ps_vp, \
                 tc.tile_pool(name="ps_e", bufs=1, space="PSUM") as ps_ep, \
                 tc.tile_pool(name="expp", bufs=4) as expp, \
                 tc.tile_pool(name="vd2p", bufs=2) as vd2p, \
                 tc.tile_pool(name="scolp", bufs=2) as scolp, \
                 tc.tile_pool(name="osbp", bufs=4) as osbp:
                # PE warm-up: ~4us of K=128 matmuls during the initial DMA
                # wait lifts the HAM clock throttle (1.2 -> 2.4 GHz) before
                # the real swaths begin; output is scratch, never read
                wps = ps_ep.tile([128, 260], F32, tag="pse")
                for wi in range(7):
                    nc.tensor.matmul(wps[:, 0:256], lhsT=WUP[:, 0:128],
                                     rhs=WUP[:], start=True, stop=True,
                                     skip_group_check=True)
                for nt in range(NT):
                    n0 = nt * TN
                    psv = ps_vp.tile([33, TN], F32)
                    vd2 = vd2p.tile([128, TN], BF16)
                    nc.vector.memset(vd2[:], 0.0)
                    m = 0
                    for sw in SW:
                        ps = ps_s.tile([128, 1536], F32)
                        ex = expp.tile([128, 1536], BF16)
                        w = sw * 512
                        for k in range(sw):
                            nc.tensor.matmul(
                                ps[:, k * 512:(k + 1) * 512],
                                lhsT=XT2[:, (m + k) * 128:(m + k + 1) * 128],
                                rhs=GP2[:, n0:n0 + TN],
                                start=True, stop=True)
                        nc.scalar.activation(ex[:, :w], ps[:, :w], AFT.Exp)
                        for k in range(sw):
                            nc.tensor.matmul(
                                psv[:],
                                lhsT=HVO[:, (m + k) * HW9:(m + k + 1) * HW9],
                                rhs=ex[:, k * 512:(k + 1) * 512],
                                start=(m + k == 0), stop=(m + k == MC - 1),
                                skip_group_check=True)
                        m += sw
                    nc.vector.tensor_copy(vd2[0:33, :], psv[:, :])
                    pse = ps_ep.tile([128, 260], F32, tag="pse")
                    scol = scolp.tile([128, 4], F32)
                    for nb in range(4):
                        nc.tensor.matmul(pse[:, 256 + nb:257 + nb],
                                         lhsT=vd2[:, nb * 128:(nb + 1) * 128],
                                         rhs=E8[:], start=True, stop=True)
                    for nb in range(4):
                        nc.tensor.matmul(pse[:, nb * 64:(nb + 1) * 64],
                                         lhsT=vd2[:, nb * 128:(nb + 1) * 128],
                                         rhs=WV2[:], start=True, stop=True)
                    nc.vector.reciprocal(scol[:], pse[:, 256:260])
                    for nb in range(4):
                        osb = osbp.tile([128, C], F32)
                        t = nt * 4 + nb
                        nc.vector.scalar_tensor_tensor(
                            osb[:], pse[:, nb * 64:(nb + 1) * 64],
                            scol[:, nb:nb + 1],
                            XRP[:, t * C:(t + 1) * C],
                            op0=AluOpType.mult, op1=AluOpType.add)
                        nc.sync.dma_start(out[t * 128:(t + 1) * 128, :], osb[:])

    nc.compile()
    return nc


_CACHE = {}


def _get_compiled():
    if "nc" not in _CACHE:
        _CACHE["nc"] = _build()
    return _CACHE["nc"]


def _make_in_maps(x, Wf, bf, Wg, bg, Wh, bh, Wv, bv, gamma):
    x = np.asarray(x, np.float32)
    Wf = np.asarray(Wf, np.float32)
    Wg = np.asarray(Wg, np.float32)
    Wh = np.asarray(Wh, np.float32)
    Wv = np.asarray(Wv, np.float32)
    bf = np.asarray(bf, np.float32)
    bg = np.asarray(bg, np.float32)
    bh = np.asarray(bh, np.float32)
    bv = np.asarray(bv, np.float32)
    g0 = float(np.asarray(gamma, np.float32).reshape(-1)[0])

    xf = x.reshape(B, N, C)
    P = Wf @ Wg.T                            # [C, C] score kernel
    wfbg = Wf @ bg                           # [C] column-bias direction
    bgbf = float(bg @ bf)
    res_bias = g0 * (bh @ Wv + bv)           # [C] folded into residual
    wv2 = np.zeros((128, C), np.float32)
    wv2[0:D] = g0 * Wv
    wv2 = wv2.astype(ml_dtypes.bfloat16)

    in_maps = []
    for i in range(NCORES):
        b, h = divmod(i, 2)
        r0 = h * RPC
        xt2 = np.zeros((128, N), np.float32)
        xt2[0:C] = xf[b].T
        xt2[C] = xf[b] @ wfbg + bgbf         # d_m: per-key score bias
        gp2 = np.zeros((128, RPC), np.float32)
        gp2[0:C] = P @ xf[b, r0:r0 + RPC].T
        gp2[C] = 1.0
        hv = xf[b] @ Wh                      # [N, D] (bh folds into res_bias)
        ho = np.zeros((MC, 128, HW9), np.float32)
        ho[:, :, 0:D] = hv.reshape(MC, 128, D)
        ho[:, :, 32] = 1.0
        ho = np.ascontiguousarray(ho.transpose(1, 0, 2).reshape(128, -1))
        xr = xf[b, r0:r0 + RPC] + res_bias   # [RPC, C]
        xrp = np.ascontiguousarray(
            xr.reshape(RPC // 128, 128, C).transpose(1, 0, 2).reshape(128, -1))
        in_maps.append({"xt2": xt2.astype(ml_dtypes.bfloat16),
                        "gp2": gp2.astype(ml_dtypes.bfloat16),
                        "hvo": ho.astype(ml_dtypes.bfloat16),
                        "xrp": xrp, "wv2": wv2})
    return in_maps


def _assemble(results):
    outf = np.empty((B, N, C), np.float32)
    for i in range(NCORES):
        b, h = divmod(i, 2)
        outf[b, h * RPC:(h + 1) * RPC] = results[i]["out"]
    return outf.reshape(B, HH, WW, C)


def run(inputs, **spmd_kwargs):
    """Returns (output, BassKernelResults)."""
    nc = _get_compiled()
    in_maps = _make_in_maps(**inputs)
    res = run_bass_kernel_spmd(nc, in_maps, core_ids=list(range(NCORES)),
                               **spmd_kwargs)
    return _assemble(res.results), res


def kernel(**inputs):
    out, _ = run(inputs)
    return out


# revision 23
# speedup vs baseline: 1.0348x; 1.0107x over previous
# SAGAN self-attention (B=4, H=W=64, C=64, D=8) on 8 TRN2 NeuronCores.
#
# Sharding: core i = (batch b=i//2, half h=i%2). Each core computes rows
# [h*2048, (h+1)*2048) of the 4096x4096 attention for its batch, fully fused
# in SBUF (no NxN matrix ever touches HBM, no collectives).
#
# Scores are computed TRANSPOSED, sT[m, n] (keys m on partitions, queries n
# on free axis) directly from x: s = x @ (Wf Wg^T) @ x^T, so the contraction
# is K=64 (channels) rather than K=8 (qk dim). The contraction is then
# zero-padded to K=128: the PE clock-gate (HAM) only counts the PE as
# "busy" when all 128 rows are active, so K<128 matmuls run at the cold
# 1.2 GHz clock forever (measured: 629 ns vs 379 ns per 512-col matmul).
# Pad row 64 carries the bias term bg.(f_m + bf) (with a ones-row in the
# moving operand), which softmax needs; the row-constant term g_n.bf is
# softmax-invariant and dropped.
#
# exp(sT) feeds the PV matmul directly as the moving operand. The PV
# stationary is [hv(8) | 0.. | 1@32], so one accumulating K=128 matmul
# produces v_unnorm^T [8, n] AND the softmax denominator on PSUM partition
# 32 (a legal 32-aligned read base). The epilogue packs v_un^T (rows 0-7)
# and the denominator (row 32) into one zero-padded K=128 stationary VD2:
# rhs=WV2 (Wv rows 0-7) gives o_un, rhs=E8 (one-hot row 32) transposes the
# denominator to a [128,1] column for the reciprocal. o = o_un/denom fused
# with the residual add as one scalar_tensor_tensor per [128, 64] tile.
#
# Host precomputes the tiny projections (0.6% of FLOPs) and folds:
# P = Wf@Wg^T into the score form, gamma into Wv, gamma*(bh@Wv+bv) into the
# residual. Attention matmuls use bf16 operands (fp32/f32r moving operands
# stream at 2-3 cycles/column vs 1 for bf16); PSUM accumulation is fp32.

import numpy as np
import ml_dtypes

import concourse.bacc as bacc
import concourse.tile as tile
import concourse.mybir as mybir
from concourse.alu_op_type import AluOpType
from concourse.bass_utils import run_bass_kernel_spmd

F32 = mybir.dt.float32
BF16 = mybir.dt.bfloat16
AFT = mybir.ActivationFunctionType

B, HH, WW, C = 4, 64, 64, 64
N = HH * WW          # 4096 sequence positions per batch
D = 8                # qkv channel dim
RPC = N // 2         # rows per core (2048)
NCORES = 8
HW9 = 33             # PV stationary width: [hv(8) | zeros(24) | ones(1)]
MC = N // 128        # 32 key chunks of 128


def _build():
    nc = bacc.Bacc("TRN2", target_bir_lowering=False, debug=False,
                   num_devices=NCORES)

    xt2 = nc.dram_tensor("xt2", [128, N], BF16, kind="ExternalInput").ap()
    gp2 = nc.dram_tensor("gp2", [128, RPC], BF16, kind="ExternalInput").ap()
    hvo = nc.dram_tensor("hvo", [128, MC * HW9], BF16,
                         kind="ExternalInput").ap()
    xrp = nc.dram_tensor("xrp", [128, RPC // 128 * C], F32,
                         kind="ExternalInput").ap()
    wv2 = nc.dram_tensor("wv2", [128, C], BF16, kind="ExternalInput").ap()
    out = nc.dram_tensor("out", [RPC, C], F32, kind="ExternalOutput").ap()

    NT = 4            # n-tiles of 512 query rows each
    TN = 512          # queries per n-tile
    SW = [3] * 10 + [2]   # swath sizes (in key chunks) per n-tile

    with tile.TileContext(nc) as tc:
        with tc.tile_pool(name="const", bufs=1) as const:
            XT2 = const.tile([128, N], BF16)
            GP2 = const.tile([128, RPC], BF16)
            HVO = const.tile([128, MC * HW9], BF16)
            XRP = const.tile([128, RPC // 128 * C], F32)
            WV2 = const.tile([128, C], BF16)
            E8 = const.tile([128, 1], BF16)
            PRE = const.tile([1, 1], F32)
            WUP = const.tile([128, 256], BF16)

            # interleave input DMAs in first-use order: swath s of n-tile 0
            # needs XT2 chunks, GP2 cols 0:512, and HVO chunks as m advances
            nc.sync.dma_start(GP2[:, 0:512], gp2[:, 0:512])
            w8 = MC * HW9 // 8
            for j in range(8):
                nc.sync.dma_start(XT2[:, j * 512:(j + 1) * 512],
                                  xt2[:, j * 512:(j + 1) * 512])
                nc.sync.dma_start(HVO[:, j * w8:(j + 1) * w8],
                                  hvo[:, j * w8:(j + 1) * w8])
            for j in range(1, 4):
                nc.sync.dma_start(GP2[:, j * 512:(j + 1) * 512],
                                  gp2[:, j * 512:(j + 1) * 512])
            nc.sync.dma_start(WV2[:], wv2[:])
            nc.sync.dma_start(XRP[:], xrp[:])
            nc.vector.memset(WUP[:], 0.0)
            nc.vector.memset(E8[:], 0.0)
            nc.vector.memset(E8[32:33, :], 1.0)
            # dummy exp: hoists the one-time ACT table load (~1.3us) into the
            # initial DMA wait instead of the first swath's critical path
            nc.scalar.activation(PRE[:], E8[0:1, :], AFT.Exp)

            with tc.tile_pool(name="ps_s", bufs=2, space="PSUM") as ps_s, \
                 tc.tile_pool(name="ps_v", bufs=1, space="PSUM") as ps_vp, \
                 tc.tile_pool(name="ps_e", bufs=1, space="PSUM") as ps_ep, \
                 tc.tile_pool(name="expp", bufs=4) as expp, \
                 tc.tile_pool(name="vd2p", bufs=2) as vd2p, \
                 tc.tile_pool(name="scolp", bufs=2) as scolp, \
                 tc.tile_pool(name="osbp", bufs=4) as osbp:
                # PE warm-up: ~4us of K=128 matmuls during the initial DMA
                # wait lifts the HAM clock throttle (1.2 -> 2.4 GHz) before
                # the real swaths begin; output is scratch, never read
                wps = ps_ep.tile([128, 260], F32, tag="pse")
                for wi in range(16):
                    nc.tensor.matmul(wps[:, 0:256], lhsT=WUP[:, 0:128],
                                     rhs=WUP[:], start=True, stop=True,
                                     skip_group_check=True)
                for nt in range(NT):
                    n0 = nt * TN
                    psv = ps_vp.tile([33, TN], F32)
                    vd2 = vd2p.tile([128, TN], BF16)
                    nc.vector.memset(vd2[:], 0.0)
                    m = 0
                    for sw in SW:
                        ps = ps_s.tile([128, 1536], F32)
                        ex = expp.tile([128, 1536], BF16)
                        w = sw * 512
                        for k in range(sw):
                            nc.tensor.matmul(
                                ps[:, k * 512:(k + 1) * 512],
                                lhsT=XT2[:, (m + k) * 128:(m + k + 1) * 128],
                                rhs=GP2[:, n0:n0 + TN],
                                start=True, stop=True)
                        nc.scalar.activation(ex[:, :w], ps[:, :w], AFT.Exp)
                        for k in range(sw):
                            nc.tensor.matmul(
                                psv[:],
                                lhsT=HVO[:, (m + k) * HW9:(m + k + 1) * HW9],
                                rhs=ex[:, k * 512:(k + 1) * 512],
                                start=(m + k == 0), stop=(m + k == MC - 1),
                                skip_group_check=True)
                        m += sw
                    nc.vector.tensor_copy(vd2[0:33, :], psv[:, :])
                    pse = ps_ep.tile([128, 260], F32, tag="pse")
                    scol = scolp.tile([128, 4], F32)
                    for nb in range(4):
                        nc.tensor.matmul(pse[:, 256 + nb:257 + nb],
                                         lhsT=vd2[:, nb * 128:(nb + 1) * 128],
                                         rhs=E8[:], start=True, stop=True)
                    for nb in range(4):
                        nc.tensor.matmul(pse[:, nb * 64:(nb + 1) * 64],
                                         lhsT=vd2[:, nb * 128:(nb + 1) * 128],
                                         rhs=WV2[:], start=True, stop=True)
                    nc.vector.reciprocal(scol[:], pse[:, 256:260])
                    for nb in range(4):
                        osb = osbp.tile([128, C], F32)
                        t = nt * 4 + nb
                        nc.vector.scalar_tensor_tensor(
                            osb[:], pse[:, nb * 64:(nb + 1) * 64],
                            scol[:, nb:nb + 1],
                            XRP[:, t * C:(t + 1) * C],
                            op0=AluOpType.mult, op1=AluOpType.add)
                        nc.sync.dma_start(out[t * 128:(t + 1) * 128, :], osb[:])

    nc.compile()
    return nc


_CACHE = {}


def _get_compiled():
    if "nc" not in _CACHE:
        _CACHE["nc"] = _build()
    return _CACHE["nc"]


def _make_in_maps(x, Wf, bf, Wg, bg, Wh, bh, Wv, bv, gamma):
    x = np.asarray(x, np.float32)
    Wf = np.asarray(Wf, np.float32)
    Wg = np.asarray(Wg, np.float32)
    Wh = np.asarray(Wh, np.float32)
    Wv = np.asarray(Wv, np.float32)
    bf = np.asarray(bf, np.float32)
    bg = np.asarray(bg, np.float32)
    bh = np.asarray(bh, np.float32)
    bv = np.asarray(bv, np.float32)
    g0 = float(np.asarray(gamma, np.float32).reshape(-1)[0])

    xf = x.reshape(B, N, C)
    P = Wf @ Wg.T                            # [C, C] score kernel
    wfbg = Wf @ bg                           # [C] column-bias direction
    bgbf = float(bg @ bf)
    res_bias = g0 * (bh @ Wv + bv)           # [C] folded into residual
    wv2 = np.zeros((128, C), np.float32)
    wv2[0:D] = g0 * Wv
    wv2 = wv2.astype(ml_dtypes.bfloat16)

    in_maps = []
    for i in range(NCORES):
        b, h = divmod(i, 2)
        r0 = h * RPC
        xt2 = np.zeros((128, N), np.float32)
        xt2[0:C] = xf[b].T
        xt2[C] = xf[b] @ wfbg + bgbf         # d_m: per-key score bias
        gp2 = np.zeros((128, RPC), np.float32)
        gp2[0:C] = P @ xf[b, r0:r0 + RPC].T
        gp2[C] = 1.0
        hv = xf[b] @ Wh                      # [N, D] (bh folds into res_bias)
        ho = np.zeros((MC, 128, HW9), np.float32)
        ho[:, :, 0:D] = hv.reshape(MC, 128, D)
        ho[:, :, 32] = 1.0
        ho = np.ascontiguousarray(ho.transpose(1, 0, 2).reshape(128, -1))
        xr = xf[b, r0:r0 + RPC] + res_bias   # [RPC, C]
        xrp = np.ascontiguousarray(
            xr.reshape(RPC // 128, 128, C).transpose(1, 0, 2).reshape(128, -1))
        in_maps.append({"xt2": xt2.astype(ml_dtypes.bfloat16),
                        "gp2": gp2.astype(ml_dtypes.bfloat16),
                        "hvo": ho.astype(ml_dtypes.bfloat16),
                        "xrp": xrp, "wv2": wv2})
    return in_maps


def _assemble(results):
    outf = np.empty((B, N, C), np.float32)
    for i in range(NCORES):
        b, h = divmod(i, 2)
        outf[b, h * RPC:(h + 1) * RPC] = results[i]["out"]
    return outf.reshape(B, HH, WW, C)


def run(inputs, **spmd_kwargs):
    """Returns (output, BassKernelResults)."""
    nc = _get_compiled()
    in_maps = _make_in_maps(**inputs)
    res = run_bass_kernel_spmd(nc, in_maps, core_ids=list(range(NCORES)),
                               **spmd_kwargs)
    return _assemble(res.results), res


def kernel(**inputs):
    out, _ = run(inputs)
    return out
